# revision 9
# baseline (speedup 1.0000x reference)
"""MultiHeadTimeDimensionAttention kernel for Trainium2 (8 NeuronCores).

Math (per batch b):
  q[h,d]      = o_last[b] . Wq[h,:,d] + bq[h,d]
  scores[t,h] = sum_z o_all[b,t,z] * wkq[z,h]        (wkq[z,h] = sum_d Wk[h,z,d] q[h,d])
                (bk folds to a per-head constant -> softmax invariant -> dropped)
  p = exp(scores - max_t), l = sum_t p               (softmax unnormalized)
  r[h,z]      = sum_t p[t,h] * o_all[b,t,z]
  ctx[h,d]    = (sum_z r[h,z] Wv[h,z,d]) / l[h] + bv[h,d]

This is algebraically exact vs. the reference (einsum reassociation only) and
reduces FLOPs by ~Z/H=64x vs. materializing K/V.

Sharding: data-parallel over B; each of the 8 cores handles B/8=2 batches.
"""

import numpy as np

import concourse.bacc as bacc
import concourse.tile as tile
import concourse.mybir as mybir
from concourse.bass_utils import run_bass_kernel_spmd
from concourse.masks import make_identity

B, T, Z, H, DK = 16, 4096, 1024, 16, 64
P = 128
NCORES = 8
BLOC = B // NCORES          # batches per core
ZC = Z // P                 # 8 z-chunks
NT = T // P                 # 32 t-tiles
TB = 512                    # t-block for scores pass
NTB = T // TB               # 8
NPAIR = H // 2              # 8 head-pairs
F32 = mybir.dt.float32
F32R = mybir.dt.float32r

# Matmul dtype switches (float32r = full-rate PE, reduced input mantissa).
SCORES_F32R = False
R_F32R = False
CTX_F32R = False


def _mm_dt(ap, use_f32r):
    return ap.bitcast(F32R) if use_f32r else ap


def build_nc():
    nc = bacc.Bacc(None, target_bir_lowering=False)

    # All weight-side params are pre-laid-out on host so DMAs are flat copies.
    o_all = nc.declare_dram_parameter("o_all", [BLOC, T, Z], F32, isOutput=False)
    o_lastT = nc.declare_dram_parameter("o_lastT", [P, ZC, BLOC], F32, isOutput=False)
    wq_full = nc.declare_dram_parameter("Wq_full", [P, ZC, Z], F32, isOutput=False)
    wkT = nc.declare_dram_parameter("WkT", [P, NPAIR, Z], F32, isOutput=False)
    wv_in = nc.declare_dram_parameter("Wv", [P, ZC, H, DK], F32, isOutput=False)
    bq_r = nc.declare_dram_parameter("bq_r", [P, ZC], F32, isOutput=False)
    bv_in = nc.declare_dram_parameter("bv", [H, DK], F32, isOutput=False)
    out = nc.declare_dram_parameter("out", [BLOC, Z], F32, isOutput=True)

    with tile.TileContext(nc) as tc:
        with (
            tc.tile_pool(name="const", bufs=1) as const,
            tc.tile_pool(name="small", bufs=2) as small,
        ):
            ident = const.tile([P, P], F32)
            make_identity(nc, ident)
            bv_sb = const.tile([H, DK], F32)
            nc.sync.dma_start(out=bv_sb, in_=bv_in[:])
            bqr_sb = const.tile([P, ZC], F32)
            nc.sync.dma_start(out=bqr_sb, in_=bq_r[:])

            # Zero-padded staging for PE transposes (rows >=16/64 stay zero).
            pT_pad = const.tile([P, T], F32)      # rows 0..H-1: scores^T then p^T
            r_pad = const.tile([P, Z], F32)       # rows 0..H-1: r
            ctx_pad = const.tile([P, H], F32)     # rows 0..DK-1: ctx columns
            nc.vector.memset(pT_pad[:, :], 0.0)
            nc.vector.memset(r_pad[:, :], 0.0)
            nc.vector.memset(ctx_pad[:, :], 0.0)

            wkq_sb = []  # per-batch (P, ZC, H): wkq[z,:] chunked over z
            # ---------------- prologue: q and wkq for both batches ----------
            with (
                tc.tile_pool(name="wpro", bufs=1) as wpro,
                tc.tile_pool(name="propsum", bufs=2, space="PSUM") as propsum,
            ):
                wq_sb = wpro.tile([P, ZC, Z], F32)   # [zp, zc, m]
                nc.sync.dma_start(out=wq_sb, in_=wq_full[:])
                wkT_sb = wpro.tile([P, NPAIR, Z], F32)  # [dd, pair, z]
                nc.sync.dma_start(out=wkT_sb, in_=wkT[:])
                olT_sb = wpro.tile([P, ZC, BLOC], F32)
                nc.sync.dma_start(out=olT_sb, in_=o_lastT[:])

                # q (full vector of H*DK=Z, chunked 128): q_sb[mp, mc, b]
                q_sb = wpro.tile([P, ZC, BLOC], F32)
                for mc in range(ZC):
                    qp = propsum.tile([P, BLOC], F32, tag="qp")
                    for zc in range(ZC):
                        nc.tensor.matmul(
                            qp,
                            wq_sb[:, zc, mc * P : (mc + 1) * P],
                            olT_sb[:, zc, :],
                            start=(zc == 0),
                            stop=(zc == ZC - 1),
                        )
                    nc.vector.tensor_tensor(
                        q_sb[:, mc, :],
                        qp,
                        bqr_sb[:, mc : mc + 1].to_broadcast((P, BLOC)),
                        mybir.AluOpType.add,
                    )

                for b in range(BLOC):
                    # head-pair selector columns: qsel[:, pr, 0]=[q_h(2pr);0],
                    # [:, pr, 1]=[0;q_h(2pr+1)]
                    qsel = wpro.tile([P, NPAIR, 2], F32, tag=f"qsel{b}")
                    nc.vector.memset(qsel, 0.0)
                    for pr in range(NPAIR):
                        nc.vector.tensor_copy(
                            out=qsel[0:DK, pr, 0:1], in_=q_sb[0:DK, pr, b : b + 1]
                        )
                        nc.vector.tensor_copy(
                            out=qsel[DK:P, pr, 1:2], in_=q_sb[DK:P, pr, b : b + 1]
                        )
                    wkq_b = const.tile([P, ZC, H], F32, tag=f"wkq{b}")
                    for zc in range(ZC):
                        wp = propsum.tile([P, H], F32, tag="wp")
                        for pr in range(NPAIR):
                            nc.tensor.matmul(
                                wp[:, 2 * pr : 2 * pr + 2],
                                wkT_sb[:, pr, zc * P : (zc + 1) * P],
                                qsel[:, pr, :],
                                start=True,
                                stop=True,
                            )
                        nc.any.tensor_copy(out=wkq_b[:, zc, :], in_=wp)
                    wkq_sb.append(wkq_b)

            # ---------------- main per-batch pipeline -----------------------
            with (
                tc.tile_pool(name="abuf", bufs=1) as abuf,
                tc.tile_pool(name="wvp", bufs=1) as wvp,
                tc.tile_pool(name="stage", bufs=3) as stage,
                tc.tile_pool(name="tpsum", bufs=2, space="PSUM") as tpsum,
                tc.tile_pool(name="mpsum", bufs=2, space="PSUM") as mpsum,
                tc.tile_pool(name="rpsum", bufs=1, space="PSUM") as rpsum,
            ):
                wv_sb = wvp.tile([P, ZC, H, DK], F32)  # [zp, zc, h, d]
                nc.sync.dma_start(out=wv_sb, in_=wv_in[:])

                for b in range(BLOC):
                    # Load A = o_all[b] as 8 blocks of (P, 4, Z); t = blk*512 + i*128 + p
                    a_sb = []
                    for blk in range(8):
                        a_t = abuf.tile([P, 4, Z], F32, tag=f"a{blk}")
                        nc.sync.dma_start(
                            out=a_t,
                            in_=o_all[b, blk * 512 : (blk + 1) * 512, :].rearrange(
                                "(i zp) z -> zp i z", zp=P
                            ),
                        )
                        a_sb.append(a_t)

                    # scores^T (H, T) into pT_pad rows 0..H-1
                    for tb in range(NTB):
                        sc_ps = mpsum.tile([H, TB], F32, tag="scps")
                        for zc in range(ZC):
                            at_ps = tpsum.tile([P, TB], F32, tag="atps")
                            for i in range(4):
                                nc.tensor.transpose(
                                    at_ps[:, i * P : (i + 1) * P],
                                    a_sb[tb][:, i, zc * P : (zc + 1) * P],
                                    ident,
                                )
                            at_sb = stage.tile([P, TB], F32, tag="atsb")
                            nc.any.tensor_copy(out=at_sb, in_=at_ps)
                            nc.tensor.matmul(
                                sc_ps,
                                _mm_dt(wkq_sb[b][:, zc, :], SCORES_F32R),
                                _mm_dt(at_sb[:], SCORES_F32R),
                                start=(zc == 0),
                                stop=(zc == ZC - 1),
                            )
                        nc.any.tensor_copy(
                            out=pT_pad[:H, tb * TB : (tb + 1) * TB], in_=sc_ps
                        )

                    # softmax on rows 0..H-1 (in place): p^T = exp(s^T - max)
                    mx = small.tile([H, 1], F32, tag="mx")
                    nc.vector.reduce_max(mx, pT_pad[:H, :], axis=mybir.AxisListType.X)
                    negmax = small.tile([H, 1], F32, tag="negmax")
                    nc.scalar.mul(out=negmax, in_=mx, mul=-1.0)
                    lsum = small.tile([H, 1], F32, tag="lsum")
                    nc.scalar.activation(
                        out=pT_pad[:H, :],
                        in_=pT_pad[:H, :],
                        func=mybir.ActivationFunctionType.Exp,
                        bias=negmax,
                        scale=1.0,
                        accum_out=lsum,
                    )
                    rinv = small.tile([H, 1], F32, tag="rinv")
                    nc.vector.reciprocal(rinv, lsum)

                    # p natural (t on partitions): p_sb[tp, tt, h]
                    p_sb = stage.tile([P, NT, H], F32, tag="psb")
                    for tt in range(NT):
                        pp = tpsum.tile([P, P], F32, tag="tp")
                        nc.tensor.transpose(
                            pp, pT_pad[:, tt * P : (tt + 1) * P], ident
                        )
                        nc.any.tensor_copy(out=p_sb[:, tt, :], in_=pp[:, :H])

                    # r^T? no: r (H, Z) = p^T A accumulated over t
                    r_ps = rpsum.tile([H, 2, TB], F32, tag="rps")
                    for zt in range(2):
                        for tt in range(NT):
                            blk, i = tt // 4, tt % 4
                            nc.tensor.matmul(
                                r_ps[:, zt, :],
                                _mm_dt(p_sb[:, tt, :], R_F32R),
                                _mm_dt(a_sb[blk][:, i, zt * TB : (zt + 1) * TB], R_F32R),
                                start=(tt == 0),
                                stop=(tt == NT - 1),
                            )
                    nc.any.tensor_copy(
                        out=r_pad[:H, :], in_=r_ps.rearrange("h a f -> h (a f)")
                    )

                    # r^T chunks (z on partitions): rt_sb[zp, zc, h]
                    rt_sb = stage.tile([P, ZC, H], F32, tag="rtsb")
                    for zc in range(ZC):
                        rt_ps = tpsum.tile([P, P], F32, tag="tp")
                        nc.tensor.transpose(
                            rt_ps, r_pad[:, zc * P : (zc + 1) * P], ident
                        )
                        nc.any.tensor_copy(out=rt_sb[:, zc, :], in_=rt_ps[:, :H])

                    # ctx columns: ctx_ps[d, h] = sum_z r^T[z,h] Wv[h,z,d]
                    ctx_ps = tpsum.tile([DK, H], F32, tag="tp")
                    for h in range(H):
                        for zc in range(ZC):
                            nc.tensor.matmul(
                                ctx_ps[:, h : h + 1],
                                _mm_dt(wv_sb[:, zc, h, :], CTX_F32R),
                                _mm_dt(rt_sb[:, zc, h : h + 1], CTX_F32R),
                                start=(zc == 0),
                                stop=(zc == ZC - 1),
                            )
                    nc.any.tensor_copy(out=ctx_pad[:DK, :], in_=ctx_ps)
                    ctxT_ps = tpsum.tile([H, P], F32, tag="tp")
                    nc.tensor.transpose(ctxT_ps, ctx_pad, ident)

                    out_sb = small.tile([H, DK], F32, tag="outsb")
                    nc.vector.tensor_scalar_mul(
                        out=out_sb, in0=ctxT_ps[:, :DK], scalar1=rinv
                    )
                    nc.vector.tensor_add(out=out_sb, in0=out_sb, in1=bv_sb)
                    nc.sync.dma_start(
                        out=out[b].rearrange("(h d) -> h d", h=H), in_=out_sb
                    )

    nc.finalize()
    return nc


_NC_CACHE = {}


def _get_nc():
    key = (SCORES_F32R, R_F32R, CTX_F32R)
    if key not in _NC_CACHE:
        _NC_CACHE[key] = build_nc()
    return _NC_CACHE[key]


def prep_inputs(o_all, o_last, Wk, Wv, Wq, bk, bv, bq):
    """Host-side shard + layout prep. Returns per-core input maps."""
    o_all = np.asarray(o_all, dtype=np.float32)
    o_last = np.asarray(o_last, dtype=np.float32)
    Wk = np.asarray(Wk, dtype=np.float32)
    Wv = np.asarray(Wv, dtype=np.float32)
    Wq = np.asarray(Wq, dtype=np.float32)
    bv = np.asarray(bv, dtype=np.float32)
    bq = np.asarray(bq, dtype=np.float32)

    # [zp, zc, m] = Wq_full[zc*128+zp, m] with Wq_full[z, h*64+d] = Wq[h,z,d]
    wq_flat = Wq.transpose(1, 0, 2).reshape(Z, Z)
    wq_full = np.ascontiguousarray(wq_flat.reshape(ZC, P, Z).transpose(1, 0, 2))
    # [dd, pr, z] = Wk[2pr + dd//64, z, dd%64]
    wkT = np.ascontiguousarray(
        Wk.transpose(0, 2, 1).reshape(NPAIR, P, Z).transpose(1, 0, 2)
    )
    # [zp, zc, h, d] = Wv[h, zc*128+zp, d]
    wv_c = np.ascontiguousarray(Wv.reshape(H, ZC, P, DK).transpose(2, 1, 0, 3))
    bq_r = np.ascontiguousarray(bq.reshape(Z).reshape(ZC, P).T)  # [P, ZC]
    bv_c = np.ascontiguousarray(bv)

    in_maps = []
    for c in range(NCORES):
        sl = slice(c * BLOC, (c + 1) * BLOC)
        # [zp, zc, b] = o_last[b, 0, zc*128+zp]
        olT = np.ascontiguousarray(
            o_last[sl, 0, :].T.reshape(ZC, P, BLOC).transpose(1, 0, 2)
        )
        in_maps.append(
            {
                "o_all": np.ascontiguousarray(o_all[sl]),
                "o_lastT": olT,
                "Wq_full": wq_full,
                "WkT": wkT,
                "Wv": wv_c,
                "bq_r": bq_r,
                "bv": bv_c,
            }
        )
    return in_maps


def kernel(o_all, o_last, Wk, Wv, Wq, bk, bv, bq, _trace=False, _trace_kwargs=None):
    nc = _get_nc()
    in_maps = prep_inputs(o_all, o_last, Wk, Wv, Wq, bk, bv, bq)
    res = run_bass_kernel_spmd(
        nc, in_maps, core_ids=list(range(NCORES)), trace=_trace,
        **(_trace_kwargs or {}),
    )
    outs = [r["out"] for r in res.results]
    full = np.concatenate(outs, axis=0).reshape(B, 1, Z)
    if _trace:
        kernel.last_result = res
    return full


# revision 12
# speedup vs baseline: 2.2909x; 2.2909x over previous
"""MultiHeadTimeDimensionAttention kernel for Trainium2 (8 NeuronCores).

Math (per batch b):
  q[h,d]      = o_last[b] . Wq[h,:,d] + bq[h,d]
  scores[t,h] = sum_z o_all[b,t,z] * wkq[z,h]        (wkq[z,h] = sum_d Wk[h,z,d] q[h,d])
                (bk folds to a per-head constant -> softmax invariant -> dropped)
  p = exp(scores - max_t), l = sum_t p               (softmax unnormalized)
  r[h,z]      = sum_t p[t,h] * o_all[b,t,z]
  ctx[h,d]    = (sum_z r[h,z] Wv[h,z,d]) / l[h] + bv[h,d]

Exact algebraic restructure of the reference (einsum reassociation), ~64x
fewer FLOPs than materializing K/V. fp16 inputs to the PE (fp32 PSUM
accumulation everywhere); softmax entirely in fp32.

Sharding: data-parallel over B; each of the 8 cores handles B/8=2 batches.
"""

import numpy as np

import concourse.bacc as bacc
import concourse.tile as tile
import concourse.mybir as mybir
from concourse.bass_utils import run_bass_kernel_spmd
from concourse.masks import make_identity

B, T, Z, H, DK = 16, 4096, 1024, 16, 64
P = 128
NCORES = 8
BLOC = B // NCORES          # batches per core
ZC = Z // P                 # 8 z-chunks
NT = T // P                 # 32 t-tiles
TB = 512                    # t-block for scores pass
NTB = T // TB               # 8
NPAIR = H // 2              # 8 head-pairs
F32 = mybir.dt.float32
F16 = mybir.dt.float16


def build_nc():
    nc = bacc.Bacc(None, target_bir_lowering=False)

    # All weight-side params are pre-laid-out (and fp16-cast) on host so
    # device DMAs are flat contiguous copies.
    o16 = nc.declare_dram_parameter("o16", [BLOC, T, Z], F16, isOutput=False)
    o_lastT = nc.declare_dram_parameter("o_lastT", [P, ZC, BLOC], F16, isOutput=False)
    wq16 = nc.declare_dram_parameter("Wq16", [P, ZC, Z], F16, isOutput=False)
    wkT16 = nc.declare_dram_parameter("WkT16", [P, NPAIR, Z], F16, isOutput=False)
    wv16 = nc.declare_dram_parameter("Wv16", [P, ZC, Z], F16, isOutput=False)
    bq_r = nc.declare_dram_parameter("bq_r", [P, ZC], F32, isOutput=False)
    bv_in = nc.declare_dram_parameter("bv", [H, DK], F32, isOutput=False)
    dmask = nc.declare_dram_parameter("dmask", [H, Z], F32, isOutput=False)
    out = nc.declare_dram_parameter("out", [BLOC, Z], F32, isOutput=True)

    with tile.TileContext(nc) as tc:
        with (
            tc.tile_pool(name="const", bufs=1) as const,
            tc.tile_pool(name="small", bufs=2) as small,
        ):
            ident = const.tile([P, P], F16)
            make_identity(nc, ident)
            identf = const.tile([P, P], F32)
            make_identity(nc, identf)
            bv_sb = const.tile([H, DK], F32)
            nc.sync.dma_start(out=bv_sb, in_=bv_in[:])
            bqr_sb = const.tile([P, ZC], F32)
            nc.sync.dma_start(out=bqr_sb, in_=bq_r[:])
            dmask_sb = const.tile([H, Z], F32)
            nc.sync.dma_start(out=dmask_sb, in_=dmask[:])

            # Zero-padded staging for PE transposes (rows >= H stay zero).
            pT_pad = const.tile([P, T], F32)      # rows 0..H-1: scores^T then p^T
            r_pad = const.tile([P, Z], F32)       # rows 0..H-1: r
            nc.vector.memset(pT_pad[:, :], 0.0)
            nc.vector.memset(r_pad[:, :], 0.0)

            wkq_sb = []  # per-batch (P, ZC, H) fp16
            # ---------------- prologue: q and wkq for both batches ----------
            with (
                tc.tile_pool(name="wpro", bufs=1) as wpro,
                tc.tile_pool(name="propsum", bufs=2, space="PSUM") as propsum,
            ):
                wq_sb = wpro.tile([P, ZC, Z], F16)   # [zp, zc, m]
                nc.sync.dma_start(out=wq_sb, in_=wq16[:])
                wkT_sb = wpro.tile([P, NPAIR, Z], F16)  # [dd, pair, z]
                nc.sync.dma_start(out=wkT_sb, in_=wkT16[:])
                olT_sb = wpro.tile([P, ZC, BLOC], F16)
                nc.sync.dma_start(out=olT_sb, in_=o_lastT[:])

                # q (full vector of H*DK=Z, chunked 128): q_sb[mp, mc, b] fp32
                q_sb = wpro.tile([P, ZC, BLOC], F32)
                for mc in range(ZC):
                    qp = propsum.tile([P, BLOC], F32, tag="qp")
                    for zc in range(ZC):
                        nc.tensor.matmul(
                            qp,
                            wq_sb[:, zc, mc * P : (mc + 1) * P],
                            olT_sb[:, zc, :],
                            start=(zc == 0),
                            stop=(zc == ZC - 1),
                        )
                    nc.vector.tensor_tensor(
                        q_sb[:, mc, :],
                        qp,
                        bqr_sb[:, mc : mc + 1].to_broadcast((P, BLOC)),
                        mybir.AluOpType.add,
                    )

                for b in range(BLOC):
                    # head-pair selector columns (fp16): qsel[:, pr, 0]=[q_h(2pr);0]
                    qsel = wpro.tile([P, NPAIR, 2], F16, tag=f"qsel{b}")
                    nc.vector.memset(qsel, 0.0)
                    for pr in range(NPAIR):
                        nc.vector.tensor_copy(
                            out=qsel[0:DK, pr, 0:1], in_=q_sb[0:DK, pr, b : b + 1]
                        )
                        nc.vector.tensor_copy(
                            out=qsel[DK:P, pr, 1:2], in_=q_sb[DK:P, pr, b : b + 1]
                        )
                    wkq_b = const.tile([P, ZC, H], F16, tag=f"wkq{b}")
                    for zc in range(ZC):
                        wp = propsum.tile([P, H], F32, tag="wp")
                        for pr in range(NPAIR):
                            nc.tensor.matmul(
                                wp[:, 2 * pr : 2 * pr + 2],
                                wkT_sb[:, pr, zc * P : (zc + 1) * P],
                                qsel[:, pr, :],
                                start=True,
                                stop=True,
                            )
                        nc.any.tensor_copy(out=wkq_b[:, zc, :], in_=wp)
                    wkq_sb.append(wkq_b)

            # ---------------- main per-batch pipeline -----------------------
            with (
                tc.tile_pool(name="abuf", bufs=1) as abuf,
                tc.tile_pool(name="wvp", bufs=1) as wvp,
                tc.tile_pool(name="stage", bufs=3) as stage,
                tc.tile_pool(name="tpsum", bufs=2, space="PSUM") as tpsum,
                tc.tile_pool(name="mpsum", bufs=2, space="PSUM") as mpsum,
                tc.tile_pool(name="rpsum", bufs=1, space="PSUM") as rpsum,
            ):
                wv_sb = wvp.tile([P, ZC, Z], F16)  # [zp, zc, h*64+d]
                nc.sync.dma_start(out=wv_sb, in_=wv16[:])

                for b in range(BLOC):
                    # A (fp16) as 8 blocks of (P, 4, Z); t = blk*512 + i*128 + p
                    a_sb = []
                    for blk in range(8):
                        a_t = abuf.tile([P, 4, Z], F16, tag=f"a{blk}")
                        nc.sync.dma_start(
                            out=a_t,
                            in_=o16[b, blk * 512 : (blk + 1) * 512, :].rearrange(
                                "(i zp) z -> zp i z", zp=P
                            ),
                        )
                        a_sb.append(a_t)

                    # scores^T (H, T) into pT_pad rows 0..H-1 (fp32)
                    for tb in range(NTB):
                        sc_ps = mpsum.tile([H, TB], F32, tag="scps")
                        for zc in range(ZC):
                            at_ps = tpsum.tile([P, TB], F16, tag="atps")
                            for i in range(4):
                                nc.tensor.transpose(
                                    at_ps[:, i * P : (i + 1) * P],
                                    a_sb[tb][:, i, zc * P : (zc + 1) * P],
                                    ident,
                                )
                            at16 = stage.tile([P, TB], F16, tag="at16")
                            nc.any.tensor_copy(out=at16, in_=at_ps)
                            nc.tensor.matmul(
                                sc_ps,
                                wkq_sb[b][:, zc, :],
                                at16[:],
                                start=(zc == 0),
                                stop=(zc == ZC - 1),
                            )
                        nc.any.tensor_copy(
                            out=pT_pad[:H, tb * TB : (tb + 1) * TB], in_=sc_ps
                        )

                    # softmax rows 0..H-1 in place: p^T = exp(s^T - max)
                    mx = small.tile([H, 1], F32, tag="mx")
                    nc.vector.reduce_max(mx, pT_pad[:H, :], axis=mybir.AxisListType.X)
                    negmax = small.tile([H, 1], F32, tag="negmax")
                    nc.scalar.mul(out=negmax, in_=mx, mul=-1.0)
                    lsum = small.tile([H, 1], F32, tag="lsum")
                    nc.scalar.activation(
                        out=pT_pad[:H, :],
                        in_=pT_pad[:H, :],
                        func=mybir.ActivationFunctionType.Exp,
                        bias=negmax,
                        scale=1.0,
                        accum_out=lsum,
                    )
                    rinv = small.tile([H, 1], F32, tag="rinv")
                    nc.vector.reciprocal(rinv, lsum)

                    # p natural (t on partitions), fp16: p_sb[tp, tt, h]
                    p_sb = stage.tile([P, NT, H], F16, tag="psb")
                    for tt in range(NT):
                        pp = tpsum.tile([P, P], F32, tag="tp")
                        nc.tensor.transpose(
                            pp, pT_pad[:, tt * P : (tt + 1) * P], identf
                        )
                        nc.any.tensor_copy(out=p_sb[:, tt, :], in_=pp[:, :H])

                    # r (H, Z) = p^T A accumulated over t (fp32 psum)
                    r_ps = rpsum.tile([H, 2, TB], F32, tag="rps")
                    for zt in range(2):
                        for tt in range(NT):
                            blk, i = tt // 4, tt % 4
                            nc.tensor.matmul(
                                r_ps[:, zt, :],
                                p_sb[:, tt, :],
                                a_sb[blk][:, i, zt * TB : (zt + 1) * TB],
                                start=(tt == 0),
                                stop=(tt == NT - 1),
                            )
                    nc.any.tensor_copy(
                        out=r_pad[:H, :], in_=r_ps.rearrange("h a f -> h (a f)")
                    )

                    # r^T chunks (z on partitions) fp16: rt_sb[zp, zc, h]
                    rt_sb = stage.tile([P, ZC, H], F16, tag="rtsb")
                    for zc in range(ZC):
                        rt_ps = tpsum.tile([P, P], F32, tag="tp")
                        nc.tensor.transpose(
                            rt_ps, r_pad[:, zc * P : (zc + 1) * P], identf
                        )
                        nc.any.tensor_copy(out=rt_sb[:, zc, :], in_=rt_ps[:, :H])

                    # ctx_full[h', m] = sum_z r[h',z] WvF[z, m]; diag blocks kept
                    cf_ps = rpsum.tile([H, 2, TB], F32, tag="rps")
                    for mt in range(2):
                        for zc in range(ZC):
                            nc.tensor.matmul(
                                cf_ps[:, mt, :],
                                rt_sb[:, zc, :],
                                wv_sb[:, zc, mt * TB : (mt + 1) * TB],
                                start=(zc == 0),
                                stop=(zc == ZC - 1),
                            )
                    masked = small.tile([H, Z], F32, tag="masked")
                    nc.vector.tensor_tensor(
                        masked,
                        cf_ps.rearrange("h a f -> h (a f)"),
                        dmask_sb,
                        mybir.AluOpType.mult,
                    )
                    ctx_sb = small.tile([H, DK], F32, tag="ctxsb")
                    nc.vector.reduce_sum(
                        ctx_sb,
                        masked.rearrange("h (g d) -> h d g", d=DK),
                        axis=mybir.AxisListType.X,
                    )

                    out_sb = small.tile([H, DK], F32, tag="outsb")
                    nc.vector.tensor_scalar_mul(
                        out=out_sb, in0=ctx_sb, scalar1=rinv
                    )
                    nc.vector.tensor_add(out=out_sb, in0=out_sb, in1=bv_sb)
                    nc.sync.dma_start(
                        out=out[b].rearrange("(h d) -> h d", h=H), in_=out_sb
                    )

    nc.finalize()
    return nc


_NC_CACHE = {}


def _get_nc():
    if "nc" not in _NC_CACHE:
        _NC_CACHE["nc"] = build_nc()
    return _NC_CACHE["nc"]


def prep_inputs(o_all, o_last, Wk, Wv, Wq, bk, bv, bq):
    """Host-side shard + layout prep. Returns per-core input maps."""
    o_all = np.asarray(o_all, dtype=np.float32)
    o_last = np.asarray(o_last, dtype=np.float32)
    Wk = np.asarray(Wk, dtype=np.float32)
    Wv = np.asarray(Wv, dtype=np.float32)
    Wq = np.asarray(Wq, dtype=np.float32)
    bv = np.asarray(bv, dtype=np.float32)
    bq = np.asarray(bq, dtype=np.float32)

    # [zp, zc, m] = Wq[h, zc*128+zp, d] with m = h*64+d
    wq_flat = Wq.transpose(1, 0, 2).reshape(Z, Z)
    wq16 = np.ascontiguousarray(
        wq_flat.reshape(ZC, P, Z).transpose(1, 0, 2)
    ).astype(np.float16)
    # [dd, pr, z] = Wk[2pr + dd//64, z, dd%64]
    wkT16 = np.ascontiguousarray(
        Wk.transpose(0, 2, 1).reshape(NPAIR, P, Z).transpose(1, 0, 2)
    ).astype(np.float16)
    # [zp, zc, h*64+d] = Wv[h, zc*128+zp, d]
    wv_flat = Wv.transpose(1, 0, 2).reshape(Z, Z)
    wv16 = np.ascontiguousarray(
        wv_flat.reshape(ZC, P, Z).transpose(1, 0, 2)
    ).astype(np.float16)
    bq_r = np.ascontiguousarray(bq.reshape(Z).reshape(ZC, P).T)  # [P, ZC]
    bv_c = np.ascontiguousarray(bv)
    dmask = np.zeros((H, Z), dtype=np.float32)
    for h in range(H):
        dmask[h, h * DK : (h + 1) * DK] = 1.0

    in_maps = []
    for c in range(NCORES):
        sl = slice(c * BLOC, (c + 1) * BLOC)
        olT16 = np.ascontiguousarray(
            o_last[sl, 0, :].T.reshape(ZC, P, BLOC).transpose(1, 0, 2)
        ).astype(np.float16)
        in_maps.append(
            {
                "o16": o_all[sl].astype(np.float16),
                "o_lastT": olT16,
                "Wq16": wq16,
                "WkT16": wkT16,
                "Wv16": wv16,
                "bq_r": bq_r,
                "bv": bv_c,
                "dmask": dmask,
            }
        )
    return in_maps


def kernel(o_all, o_last, Wk, Wv, Wq, bk, bv, bq, _trace=False, _trace_kwargs=None):
    nc = _get_nc()
    in_maps = prep_inputs(o_all, o_last, Wk, Wv, Wq, bk, bv, bq)
    res = run_bass_kernel_spmd(
        nc, in_maps, core_ids=list(range(NCORES)), trace=_trace,
        **(_trace_kwargs or {}),
    )
    outs = [r["out"] for r in res.results]
    full = np.concatenate(outs, axis=0).reshape(B, 1, Z)
    if _trace:
        kernel.last_result = res
    return full
